# revision 8
# baseline (speedup 1.0000x reference)
# GAT (2-layer, 8-head) Trainium2 Bass kernel, v2.
# Data-parallel over batch across 8 NeuronCores (2 batches/core).
#
# Score stage avoids the ACT exp entirely: softmax weights are computed as
# f16 BIT PATTERNS via a Schraudolph-style trick. With A = 1024*log2(e) and
# B = f16-bits of 2^-7, uint16(A*leaky(u) + B) bitcast to f16 approximates
# exp(leaky(u))*2^-7 (global scale cancels in softmax). leaky comes either
# from ACT Prelu (reading the broadcast s1 row directly, per-head s2 as the
# per-partition bias) or from a DVE max(t, 0.2t+0.8B) pair. Masking is a
# multiplicative 0/1 f16 tile precomputed on host (transposed), applied in
# the same fused op that adds B. exp never runs on any engine; there are no
# ACT table switches.
#
# Scores are built transposed ([j partitions, i free]) so attn @ P needs no
# transpose; softmax denominator comes from a ones-column in the P operand;
# division is applied to the tiny output. The uint16 v tiles feed the PE
# matmul through a f16 bitcast.
import os
import numpy as np
from contextlib import ExitStack

LN_EPS = 1e-5
A2 = 1024.0 / float(np.log(2.0))  # f16 bits per e-fold
BP = 8192.0 - 6.0                 # bits of 2^-7, Schraudolph-calibrated
MT_NEG = 65000.0                  # additive mask magnitude (f16-safe)

_CACHE = {}
LAST_EXEC_NS = None
NSDVE_DEFAULT = int(os.environ.get("GAT_NSDVE", "2"))
NPOOL_DEFAULT = int(os.environ.get("GAT_NPOOL", "2"))


def _bcast_ap(ap, p=128):
    """Replicate a [free...] AP across p partitions (stride-0 partition dim)."""
    import concourse.bass as bass
    return bass.AP(tensor=ap.tensor, offset=ap.offset, ap=[[0, p]] + list(ap.ap))


def _build(B2, S, M, H, L, semantic, apply_g, reps=1,
           nsdve=NSDVE_DEFAULT, npool=NPOOL_DEFAULT):
    import concourse.bass as bass
    import concourse.bacc as bacc
    import concourse.tile as tile
    from concourse import mybir
    from concourse._compat import axon_active

    f16 = mybir.dt.float16
    f32 = mybir.dt.float32
    u16 = mybir.dt.uint16
    Alu = mybir.AluOpType
    Act = mybir.ActivationFunctionType

    DK = M // H
    ST = S // 128          # row tiles (also column tiles)
    KT = M // 128          # contraction tiles for the projection
    HC = H * 36            # packed cols/head: 32 P, 1 one, A2*s2, A2*s1, A2*s2+BP
    CH = min(4, ST)        # j-tiles per dense chunk
    NCH = ST // CH

    # head -> path: "sd" (DVE schraudolph), "sp" (ACT prelu + Pool pack),
    # "sa" (ACT prelu + DVE pack)
    path = ["sa"] * H
    for i, h in enumerate([2, 6, 3, 7]):
        if i < nsdve:
            path[h] = "sd"
    for i, h in enumerate([0, 4, 1, 5]):
        if i < npool:
            path[h] = "sp"

    nc = bacc.Bacc(
        "TRN2", target_bir_lowering=False, debug=not axon_active(), num_devices=8)
    keep_d = nc.declare_dram_parameter("mbpT", [B2, S, S], f16, isOutput=False)
    if semantic:
        keep2_d = nc.declare_dram_parameter("mbpT2", [B2, S, S], f16, isOutput=False)
    x0_d = nc.declare_dram_parameter("x0", [B2, S, M], f32, isOutput=False)
    pw_d = nc.declare_dram_parameter("pwcat", [L, KT, 128, HC], f16, isOutput=False)
    bc_d = nc.declare_dram_parameter("biascat", [L, HC], f32, isOutput=False)
    if apply_g:
        g_d = nc.declare_dram_parameter("lng", [L, M], f32, isOutput=False)
        b_d = nc.declare_dram_parameter("lnb", [L, M], f32, isOutput=False)
    out_d = nc.declare_dram_parameter("out", [B2, S, M], f32, isOutput=True)

    with tile.TileContext(nc) as tc, ExitStack() as ctx:
        singles = ctx.enter_context(tc.tile_pool(name="singles", bufs=1))
        persist = ctx.enter_context(tc.tile_pool(name="persist", bufs=1))
        io = ctx.enter_context(tc.tile_pool(name="io", bufs=2))
        dense = ctx.enter_context(tc.tile_pool(name="dense", bufs=2))
        xpool = ctx.enter_context(tc.tile_pool(name="xpool", bufs=4))
        lay = ctx.enter_context(tc.tile_pool(name="lay", bufs=2))
        small = ctx.enter_context(tc.tile_pool(name="small", bufs=2))
        pprojp = ctx.enter_context(tc.tile_pool(name="pprojp", bufs=2, space="PSUM"))
        dramp = ctx.enter_context(tc.tile_pool(name="dramp", bufs=2, space="DRAM"))
        pavp = ctx.enter_context(tc.tile_pool(name="pavp", bufs=4, space="PSUM"))

        eps_t = singles.tile([128, 1], f32)
        nc.vector.memset(eps_t[:], LN_EPS)

        rep_cm = tc.For_i(
            0, reps, 1, name="rep",
            hint_engines=(mybir.EngineType.PE, mybir.EngineType.DVE,
                          mybir.EngineType.Activation, mybir.EngineType.SP,
                          mybir.EngineType.Pool)) if reps > 1 else None
        if rep_cm is not None:
            ctx.enter_context(rep_cm)

        # -------- masks: DMA transposed additive post-Prelu mask (f16) -------
        # mbp[j,i] = BP where kept, -65504 where masked (so w' + mbp < 0 for
        # every masked lane and the uint16 cast saturates it to +0.0).
        keeps = []   # keeps[b] = list of variant mbp tiles; mts[b] additive pre
        mts = []
        for b in range(B2):
            kvars = []
            srcs = [keep_d] + ([keep2_d] if semantic else [])
            for vi, src in enumerate(srcs):
                kt_ = persist.tile([128, ST, S], f16, tag=f"keep{b}_{vi}",
                                   name=f"keep{b}_{vi}")
                for jt in range(ST):
                    nc.sync.dma_start(
                        out=kt_[:, jt, :], in_=src[b, jt * 128:(jt + 1) * 128, :])
                kvars.append(kt_)
            keeps.append(kvars)
            if nsdve > 0:
                mvars = []
                for vi, kt_ in enumerate(kvars):
                    mt = persist.tile([128, ST, S], f16, tag=f"mt{b}_{vi}",
                                      name=f"mt{b}_{vi}")
                    # mt = (mbp < 0) * -MT_NEG  -> {-MT_NEG masked, 0 kept}
                    nc.vector.tensor_scalar(
                        out=mt[:], in0=kt_[:], scalar1=0.0, scalar2=-MT_NEG,
                        op0=Alu.is_lt, op1=Alu.mult)
                    mvars.append(mt)
                mts.append(mvars)

        # ---------------- x0 load & cast ----------------
        xf16 = {}
        for b in range(B2):
            xf16[(b, 0)] = xpool.tile([128, ST, M], f16, tag="xf16", name=f"xf16_{b}_0")
            for s in range(ST):
                xs = io.tile([128, M], f32, tag="x0s")
                nc.sync.dma_start(out=xs[:], in_=x0_d[b, s * 128:(s + 1) * 128, :])
                nc.vector.tensor_copy(out=xf16[(b, 0)][:, s, :], in_=xs[:])

        # ---------------- Layers ----------------
        for l in range(L):
            pw_sb = [lay.tile([128, HC], f16, tag="pwsb", name=f"pwsb{_}") for _ in range(KT)]
            for kt in range(KT):
                nc.sync.dma_start(out=pw_sb[kt][:], in_=pw_d[l, kt])
            biasb = lay.tile([128, HC], f32, tag="biasb")
            nc.sync.dma_start(out=biasb[:], in_=_bcast_ap(bc_d[l]))
            if apply_g:
                gb = lay.tile([128, M], f32, tag="gb")
                nc.sync.dma_start(out=gb[:], in_=_bcast_ap(g_d[l]))
                bb = lay.tile([128, M], f32, tag="bb")
                nc.sync.dma_start(out=bb[:], in_=_bcast_ap(b_d[l]))

            for b in range(B2):
                x16 = xf16[(b, l)]
                vsel = 1 if (semantic and l > 0) else 0
                keep_t = keeps[b][vsel]
                mt_t = mts[b][vsel] if nsdve > 0 else None

                # xT (f16, [m, s] layout) via DMA xbar transposes
                xT = lay.tile([128, KT, S], f16, tag="xT")
                for kt in range(KT):
                    for s in range(ST):
                        nc.sync.dma_start_transpose(
                            out=xT[:, kt, s * 128:(s + 1) * 128],
                            in_=x16[:, s, kt * 128:(kt + 1) * 128])

                # Projection: P_sb[:, s, h, :]: 32 P, 1.0, A2*s2, A2*s1, A2*s2+BP
                P_sb = lay.tile([128, ST, H, 36], f16, tag="Psb")
                for s in range(ST):
                    pproj = pprojp.tile([128, HC], f32, tag="pproj")
                    for kt in range(KT):
                        nc.tensor.matmul(
                            pproj[:], xT[:, kt, s * 128:(s + 1) * 128], pw_sb[kt][:],
                            start=(kt == 0), stop=(kt == KT - 1))
                    nc.vector.scalar_tensor_tensor(
                        out=P_sb[:, s, :, :], in0=pproj[:], scalar=0.0, in1=biasb[:],
                        op0=Alu.add, op1=Alu.add)

                # s1 (A2-scaled) lives in P_sb[:, st, h, 34]; bounce via DRAM to
                # get per-head [128, S] partition-broadcast rows.
                s1dram = dramp.tile([H, S], f16, tag="s1dram")
                for st in range(ST):
                    nc.sync.dma_start(
                        out=bass.AP(tensor=s1dram.tensor, offset=s1dram.offset + st * 128,
                                    ap=[[1, 128], [S, H]]),
                        in_=P_sb[:, st, :, 34])

                conc = lay.tile([128, ST, M], f16, tag="conc")
                for h in range(H):
                    p = path[h]
                    s1b = dense.tile([128, S], f16, tag="s1b", bufs=4)
                    nc.sync.dma_start(out=s1b[:], in_=_bcast_ap(s1dram[h, :]))
                    pav = pavp.tile([128, ST, 36], f32, tag="pav")
                    vs = []
                    for c in range(NCH):
                        z = dense.tile([128, CH, S], f16, tag="z", bufs=3)
                        v = dense.tile([128, CH, S], u16, tag="v", bufs=2 * NCH,
                                       name=f"v{c}")
                        if p == "sd":
                            # t = mt + (A2*s2 + BP) + A2*s1 ; r = 0.2t + 0.8BP
                            # v = uint16(max(t, r))  (negatives saturate to 0)
                            for j in range(CH):
                                jt = c * CH + j
                                nc.vector.scalar_tensor_tensor(
                                    out=z[:, j, :], in0=mt_t[:, jt, :],
                                    scalar=P_sb[:, jt, h, 35:36], in1=s1b[:],
                                    op0=Alu.add, op1=Alu.add)
                            r = dense.tile([128, CH, S], f16, tag="r", bufs=2)
                            nc.vector.tensor_scalar(
                                out=r[:], in0=z[:], scalar1=0.2, scalar2=0.8 * BP,
                                op0=Alu.mult, op1=Alu.add)
                            nc.vector.tensor_tensor(
                                out=v[:], in0=z[:], in1=r[:], op=Alu.max)
                        else:
                            # w' = A2*leaky(s1+s2) via ACT Prelu on the shared
                            # broadcast row; v = uint16(w' + mbp)
                            for j in range(CH):
                                jt = c * CH + j
                                nc.scalar.activation(
                                    out=z[:, j, :], in_=s1b[:], func=Act.Prelu,
                                    bias=P_sb[:, jt, h, 33:34], alpha=0.2)
                            if p == "sp":
                                z2 = dense.tile([128, CH, S], f16, tag="r", bufs=2)
                                nc.gpsimd.tensor_tensor(
                                    out=z2[:], in0=z[:],
                                    in1=keep_t[:, c * CH:(c + 1) * CH, :],
                                    op=Alu.add)
                                nc.vector.tensor_copy(out=v[:], in_=z2[:])
                            else:
                                nc.vector.tensor_tensor(
                                    out=v[:], in0=z[:],
                                    in1=keep_t[:, c * CH:(c + 1) * CH, :],
                                    op=Alu.add)
                        vs.append(v)
                    for ib in range(ST):
                        for c in range(NCH):
                            for j in range(CH):
                                jt = c * CH + j
                                nc.tensor.matmul(
                                    pav[:, ib, 0:33],
                                    vs[c][:, j, ib * 128:(ib + 1) * 128].bitcast(f16),
                                    P_sb[:, jt, h, 0:33],
                                    start=(jt == 0), stop=(jt == ST - 1))
                    rec = small.tile([128, ST], f32, tag="rec")
                    nc.vector.reciprocal(out=rec[:], in_=pav[:, :, 32])
                    nc.vector.tensor_tensor(
                        out=conc[:, :, h * DK:(h + 1) * DK],
                        in0=pav[:, :, 0:DK],
                        in1=rec[:].rearrange("p (s one) -> p s one", one=1).broadcast_to([128, ST, DK]),
                        op=Alu.mult)

                # Residual + LayerNorm
                rr = lay.tile([128, ST, M], f16, tag="rr")
                sums = small.tile([128, ST], f32, tag="sums")
                sq = small.tile([128, ST], f32, tag="sq")
                for s in range(ST):
                    nc.vector.scalar_tensor_tensor(
                        out=rr[:, s, :], in0=conc[:, s, :], scalar=0.0, in1=x16[:, s, :],
                        op0=Alu.add, op1=Alu.add, accum_out=sums[:, s:s + 1])
                    scr = small.tile([128, M], f32, tag="scr")
                    nc.scalar.activation(out=scr[:], in_=rr[:, s, :], func=Act.Square,
                                         accum_out=sq[:, s:s + 1])
                mu = small.tile([128, ST], f32, tag="mu")
                nc.vector.tensor_scalar(out=mu[:], in0=sums[:], scalar1=1.0 / M,
                                        scalar2=None, op0=Alu.mult)
                mu2 = small.tile([128, ST], f32, tag="mu2")
                nc.vector.tensor_tensor(out=mu2[:], in0=mu[:], in1=mu[:], op=Alu.mult)
                var = small.tile([128, ST], f32, tag="var")
                nc.vector.scalar_tensor_tensor(
                    out=var[:], in0=sq[:], scalar=1.0 / M, in1=mu2[:],
                    op0=Alu.mult, op1=Alu.subtract)
                # rstd via Babylonian iterations + reciprocal (no ACT tables)
                ve = small.tile([128, ST], f32, tag="ve")
                nc.vector.tensor_scalar(out=ve[:], in0=var[:], scalar1=LN_EPS,
                                        scalar2=None, op0=Alu.add)
                std = small.tile([128, ST], f32, tag="std")
                nc.vector.tensor_scalar(out=std[:], in0=ve[:], scalar1=0.4,
                                        scalar2=0.7, op0=Alu.mult, op1=Alu.add)
                for _it in range(3):
                    rs = small.tile([128, ST], f32, tag="rs", name=f"rs{_it}")
                    nc.vector.reciprocal(out=rs[:], in_=std[:])
                    tdiv = small.tile([128, ST], f32, tag="tdiv", name=f"tdiv{_it}")
                    nc.vector.tensor_tensor(out=tdiv[:], in0=ve[:], in1=rs[:],
                                            op=Alu.mult)
                    usum = small.tile([128, ST], f32, tag="usum", name=f"usum{_it}")
                    nc.vector.tensor_tensor(out=usum[:], in0=std[:], in1=tdiv[:],
                                            op=Alu.add)
                    std2 = small.tile([128, ST], f32, tag="std", name=f"std{_it}")
                    nc.vector.tensor_scalar(out=std2[:], in0=usum[:], scalar1=0.5,
                                            scalar2=None, op0=Alu.mult)
                    std = std2
                rstd = small.tile([128, ST], f32, tag="rstd")
                nc.vector.reciprocal(out=rstd[:], in_=std[:])

                last = (l == L - 1)
                if not last:
                    xf16[(b, l + 1)] = xpool.tile([128, ST, M], f16, tag="xf16",
                                                  name=f"xf16_{b}_{l+1}")
                for s in range(ST):
                    if apply_g:
                        tmp = small.tile([128, M], f32, tag="ytmp")
                        nc.vector.tensor_scalar(
                            out=tmp[:], in0=rr[:, s, :], scalar1=mu[:, s:s + 1],
                            scalar2=rstd[:, s:s + 1], op0=Alu.subtract, op1=Alu.mult)
                        tmp2 = small.tile([128, M], f32, tag="ytmp2")
                        nc.vector.tensor_tensor(out=tmp2[:], in0=tmp[:], in1=gb[:], op=Alu.mult)
                        if last:
                            y = small.tile([128, M], f32, tag="yout")
                            nc.vector.tensor_tensor(out=y[:], in0=tmp2[:], in1=bb[:], op=Alu.add)
                            nc.sync.dma_start(out=out_d[b, s * 128:(s + 1) * 128, :], in_=y[:])
                        else:
                            nc.vector.tensor_tensor(out=xf16[(b, l + 1)][:, s, :],
                                                    in0=tmp2[:], in1=bb[:], op=Alu.add)
                    else:
                        if last:
                            y = small.tile([128, M], f32, tag="yout")
                            nc.vector.tensor_scalar(
                                out=y[:], in0=rr[:, s, :], scalar1=mu[:, s:s + 1],
                                scalar2=rstd[:, s:s + 1], op0=Alu.subtract, op1=Alu.mult)
                            nc.sync.dma_start(out=out_d[b, s * 128:(s + 1) * 128, :], in_=y[:])
                        else:
                            nc.vector.tensor_scalar(
                                out=xf16[(b, l + 1)][:, s, :], in0=rr[:, s, :],
                                scalar1=mu[:, s:s + 1], scalar2=rstd[:, s:s + 1],
                                op0=Alu.subtract, op1=Alu.mult)
    nc.compile()
    return nc


def _get_nc(key):
    if key not in _CACHE:
        _CACHE[key] = _build(*key)
    return _CACHE[key]


def _pack_weights(proj_w, proj_b, attn_w, attn_b):
    L, H, M, DK = proj_w.shape
    KT = M // 128
    HC = H * 36
    pwcat = np.zeros((L, M, H, 36), np.float32)
    biascat = np.zeros((L, H, 36), np.float32)
    for l in range(L):
        a1, a2 = attn_w[l, :DK], attn_w[l, DK:]
        for h in range(H):
            pwcat[l, :, h, :32] = proj_w[l, h]
            pwcat[l, :, h, 33] = A2 * (proj_w[l, h] @ a2)
            pwcat[l, :, h, 34] = A2 * (proj_w[l, h] @ a1)
            pwcat[l, :, h, 35] = pwcat[l, :, h, 33]
            biascat[l, h, :32] = proj_b[l, h]
            biascat[l, h, 32] = 1.0
            biascat[l, h, 33] = A2 * (proj_b[l, h] @ a2)
            biascat[l, h, 34] = A2 * (proj_b[l, h] @ a1 + attn_b[l])
            biascat[l, h, 35] = biascat[l, h, 33] + BP
    return (pwcat.reshape(L, KT, 128, HC).astype(np.float16),
            biascat.reshape(L, HC))


def _prepare(adj, inputs, score_mask, type, proj_w, proj_b, attn_w, attn_b, ln_g, ln_b):
    adj = np.asarray(adj)
    inputs = np.asarray(inputs, dtype=np.float32)
    score_mask = np.asarray(score_mask)
    proj_w = np.asarray(proj_w, dtype=np.float32)
    proj_b = np.asarray(proj_b, dtype=np.float32)
    attn_w = np.asarray(attn_w, dtype=np.float32)
    attn_b = np.asarray(attn_b, dtype=np.float32)
    ln_g = np.asarray(ln_g, dtype=np.float32)
    ln_b = np.asarray(ln_b, dtype=np.float32)

    B, S, M = inputs.shape
    L, H = proj_w.shape[0], proj_w.shape[1]
    NCORES = 8
    B2 = B // NCORES
    semantic = bool(np.asarray(type) == 1)
    apply_g = not (np.allclose(ln_g, 1.0) and np.allclose(ln_b, 0.0))

    pwcat, biascat = _pack_weights(proj_w, proj_b, attn_w, attn_b)
    sm = score_mask[:, 0]
    keep = np.logical_and(adj != 0, np.logical_not(sm))

    def _mbp(k):
        kT = k.transpose(0, 2, 1)
        return np.where(kT, np.float16(BP), np.float16(-65504.0))

    mbpT = np.ascontiguousarray(_mbp(keep))
    if semantic:
        mbpT2 = np.ascontiguousarray(_mbp(np.logical_not(sm)))

    in_maps = []
    for c in range(NCORES):
        m = {
            "mbpT": mbpT[c * B2:(c + 1) * B2],
            "x0": np.ascontiguousarray(inputs[c * B2:(c + 1) * B2]),
            "pwcat": pwcat, "biascat": biascat,
        }
        if semantic:
            m["mbpT2"] = mbpT2[c * B2:(c + 1) * B2]
        if apply_g:
            m["lng"] = ln_g
            m["lnb"] = ln_b
        in_maps.append(m)

    return (B2, S, M, H, L, semantic, apply_g), in_maps


def kernel(**inputs):
    from concourse.bass_utils import run_bass_kernel_spmd
    key, in_maps = _prepare(**inputs)
    nc = _get_nc(key)
    res = run_bass_kernel_spmd(nc, in_maps, core_ids=list(range(len(in_maps))))
    global LAST_EXEC_NS
    LAST_EXEC_NS = res.exec_time_ns
    out = np.concatenate([r["out"] for r in res.results], axis=0)
    return out.astype(np.float32)
